# revision 32
# baseline (speedup 1.0000x reference)
"""Multi-head attention kernel for Trainium2, 8 NeuronCores — v3.

Sharding: data-parallel over (batch, query-half): core i handles batch i//2
and query rows (i%2)*1024 ... +1024 (no collectives; K/V projection duplicated
between the 2 cores of a batch).

Per-core dataflow, all activation tiles SBUF-resident (no DRAM scratch):
  xT   bf16 [128, 8k, 2048]       (own query-half columns first)
  K^T, Q^T: bf16 matmul + bias, requantized fp8e4, natural pair layout
        [128=(h%2)*64+d, pair, s] — the only fp8 tensors in the pipeline
  V    bf16 [128 sk, 16 t, 16 h, 66] with a ones column per head (col 64)
  scores^T[sk, sq]: fp8 DoubleRow matmul, both operands broadcast_to a
        stride-0 slot dim so the PE computes 2*K^T Q at 0.5 cycles/row;
        the factor 2 folds into the exp scale (0.0625)
  P^T  = exp(scores/16): split between ACT (Exp) and a custom DVE op
        (EXP8_MHA polynomial) at a tunable ratio (default 9:7 per 16 chunks)
  AV:  out[sq, 4 sl, 65] = P^T-tile.T @ [V|1], N=65 bf16; the unit's first
        AV matmul start=True pre-zeroes the whole psum bank
  norm: reciprocal (DVE) + per-sl scalar mul (Pool)
  transpose back to out^T via PE identity matmul; psum->SBUF copy on Pool
  y    = outT.T @ Wo^T + bo' (bo' = bo + Wo@bv host-folded), f32 out

v3 scheduling: units run half-major (all query-half-0 heads first, then
half-1) so the first half of the output projection interleaves with the
second half of the score/AV flow instead of serializing at the end. All
non-exp elementwise work (bias adds, V copies, norm muls, outT copies,
y bias) runs on the idle Pool/GPSIMD engine so ACT+DVE split exp evenly.
DMA loads are ordered x+first-half weights first so projection fills start
~immediately; wo loads into SBUF space freed by wv/wq/xq mid-flow.
"""

import os

os.environ.setdefault("MYCRO_LOCAL_CACHE", "1")

import numpy as np

_B = lambda k, d: int(os.environ.get(k, d))

try:
    import concourse.bass as bass
except ImportError:  # pragma: no cover
    import sys

    for p in ("/opt/trn_rl_repo", "/root/.axon_site/_ro/trn_rl_repo"):
        if os.path.isdir(p) and p not in sys.path:
            sys.path.insert(0, p)
    import concourse.bass as bass

import concourse.mybir as mybir
import concourse.tile as tile
from concourse import bacc, bass_utils

BF16 = mybir.dt.bfloat16
F32 = mybir.dt.float32
FP8 = mybir.dt.float8e4
AF = mybir.ActivationFunctionType
DR = mybir.MatmulPerfMode.DoubleRow

B = 4
S = 2048
DM = 1024
H = 16
HD = 64
KT = 8          # d_model contraction chunks of 128
NG = 4          # head groups of 4
NSKT = 16       # sk tiles of 128
SQ = 1024       # query rows per core
NU = 32         # units = (head, sq-half of 512)
N_CORES = 8

# quadratic p(s) ~= exp(s/64); P = p(s)^8 = exp(s/8). Minimax on |s/64|<=0.3
EXPC2, EXPC1, EXPC0 = 1.213826721968566e-04, 1.579928854091444e-02, 1.0002496992257086

_CACHE: dict = {}


def _register_exp8():
    """Register the custom DVE op EXP8_MHA (documented dve_ops extension
    point, done at runtime so kernel.py stays self-contained)."""
    import concourse.dve_ops as dve_ops
    from concourse.dve_spec import Spec, Src0, C0, C1, C2, sq as dsq
    from concourse.dve_spec import lower as dve_lower
    from concourse.dve_uop import DveOpSpec

    name = "EXP8_MHA"
    if name in dve_ops._SUB_OPCODE_FOR_NAME:
        return dve_ops._BY_NAME_EXP8

    def _ref(in0, in1, s0, s1, imm2):
        x = np.asarray(in0, np.float32)
        p = ((x * np.float32(s0) + np.float32(s1)) * x + np.float32(imm2)).astype(
            np.float32
        )
        p = (p * p).astype(np.float32)
        p = (p * p).astype(np.float32)
        p = (p * p).astype(np.float32)
        return p

    body = dsq(dsq(dsq((Src0 * C0 + C1) * Src0 + C2)))
    spec = Spec(body=body, reference=_ref)
    row = dve_ops._CUSTOM_DVE_ROW_BASE + len(dve_ops.OPS)
    shas = {}
    for ver in ("v3", "v4"):
        uops = dve_lower(spec, ver=ver)
        shas[ver] = DveOpSpec(name=name, opcode=row, uops=uops, rd1_en=False).sha(ver)
    op = dve_ops.DveOp(name, spec, subdim=False, uops_sha=shas)
    dve_ops.OPS.append(op)
    dve_ops.CUSTOM_DVE_SPECS[name] = spec
    dve_ops._SUB_OPCODE_FOR_NAME[name] = row
    dve_ops._BY_NAME_EXP8 = op
    return op


def build_program():
    EXP8 = _register_exp8()
    nc = bacc.Bacc("TRN2", target_bir_lowering=False, debug=False)

    KFP8 = _B("KFP8", 0)
    QBF16 = _B("QBF16", 0)
    xT = nc.dram_tensor("xT", [128, KT, S], BF16, kind="ExternalInput")
    wk = nc.dram_tensor("wk", [128, KT, DM], FP8 if KFP8 else BF16,
                        kind="ExternalInput")
    xq8 = nc.dram_tensor("xq8", [128, KT, S if KFP8 else SQ], FP8,
                         kind="ExternalInput")
    wq = nc.dram_tensor("wq", [128, KT, DM], BF16 if QBF16 else FP8,
                        kind="ExternalInput")
    wv = nc.dram_tensor("wv", [128, KT, DM], BF16, kind="ExternalInput")
    wo = nc.dram_tensor("wo", [128, KT, DM], BF16, kind="ExternalInput")
    bk = nc.dram_tensor("bk", [128, KT], F32, kind="ExternalInput")
    bq = nc.dram_tensor("bq", [128, KT], F32, kind="ExternalInput")
    bo2 = nc.dram_tensor("bo2", [1, DM], F32, kind="ExternalInput")
    y = nc.dram_tensor("y", [SQ, DM], F32, kind="ExternalOutput")

    NODR = _B("NODR", 0)
    ESCALE = 0.125 if NODR else 0.0625
    ACT16 = _B("ACT16", 9)   # of every 16 exp chunks, this many go to ACT
    LAG = _B("AVLAG", 10)
    # engine choice per elementwise-op class: 0 = DVE, 1 = ACT
    # (GPSIMD/Pool cannot touch PSUM, so all of these are ACT-or-DVE)
    E_BIAS = _B("EBIAS", 0)
    E_VCOPY = _B("EVCOPY", 1)
    E_NORM = _B("ENORM", 0)

    with tile.TileContext(nc) as tc:
        with tc.tile_pool(name="pers", bufs=1) as pers:
            bo2_sb = pers.tile([1, DM], F32)
            bo2h = pers.tile([1, DM], BF16)
            ones_row = pers.tile([1, 128], BF16)
            bk_sb = pers.tile([128, KT], F32)
            bq_sb = pers.tile([128, KT], F32)
            # natural pair layout: pair p = heads (2p, 2p+1); head h on
            # partitions [64*(h%2), 64*(h%2)+64)
            kt_sb = pers.tile([128, 8, S], FP8)          # 16 KiB/part
            qt_sb = pers.tile([128, 8, SQ], FP8)         # 8 KiB/part
            vsb = pers.tile([128, NSKT, H, 66], BF16)    # 32.5 KiB/part
            outT = pers.tile([128, KT, SQ], BF16)        # 16 KiB/part
            wk_sb = pers.tile([128, KT, DM], FP8 if KFP8 else BF16)

            with (
                tc.tile_pool(name="scp", bufs=_B("SCB", 3), space="PSUM") as scp,
                tc.tile_pool(name="ptp", bufs=_B("PTB", 2)) as ptp,
                tc.tile_pool(name="ntp", bufs=2) as ntp,
                tc.tile_pool(name="rcp", bufs=2) as rcp,
                tc.tile_pool(name="xp", bufs=1) as xp,
                tc.tile_pool(name="avp", bufs=2, space="PSUM") as avp,
                tc.tile_pool(name="ydp", bufs=3) as ydp,
            ):
                xt_sb = xp.tile([128, KT, S], BF16)

                pt_t, av_t, nt_t = {}, {}, {}
                chunk_ctr = [0]

                def v_steps(wv_sb, t, c0):
                    """(psum-alloc, [per-k matmul step], epilogue) for a V fill."""
                    cell = [None]

                    def alloc():
                        cell[0] = scp.tile(
                            [128, 2, 512], F32, tag="sc", name=f"v{t}_{c0}"
                        )

                    def step(k):
                        nc.tensor.matmul(
                            cell[0][:, 0, :],
                            xt_sb[:, k, 128 * t : 128 * (t + 1)],
                            wv_sb[:, k, 512 * c0 : 512 * (c0 + 1)],
                            start=(k == 0),
                            stop=(k == KT - 1),
                        )

                    def fin():
                        if E_VCOPY:
                            nc.scalar.copy(
                                vsb[:, t, 8 * c0 : 8 * (c0 + 1), 0:64],
                                cell[0][:, 0, :],
                            )
                        else:
                            nc.vector.tensor_copy(
                                vsb[:, t, 8 * c0 : 8 * (c0 + 1), 0:64],
                                cell[0][:, 0, :],
                            )

                    return alloc, step, fin

                def kq_steps(wq_sb, xq_sb, p, sh, is_q):
                    b_sb = bq_sb if is_q else bk_sb
                    dst = (qt_sb if is_q else kt_sb)[:, p, 512 * sh : 512 * (sh + 1)]
                    cell = [None]

                    def alloc():
                        cell[0] = scp.tile(
                            [128, 2, 512], F32, tag="sc",
                            name=f"{'q' if is_q else 'k'}{p}_{sh}",
                        )

                    def step(k):
                        if is_q or KFP8:
                            # fp8 DoubleRow over k-chunk pairs; emit at odd k
                            if k % 2 == 0:
                                return
                            kp = k // 2
                            w = wq_sb if is_q else wk_sb
                            nc.tensor.matmul(
                                cell[0][:, 0, :],
                                w[:, 2 * kp : 2 * kp + 2, 128 * p : 128 * (p + 1)],
                                xq_sb[:, 2 * kp : 2 * kp + 2, 512 * sh : 512 * (sh + 1)],
                                start=(kp == 0),
                                stop=(kp == KT // 2 - 1),
                                perf_mode=DR,
                            )
                        else:
                            nc.tensor.matmul(
                                cell[0][:, 0, :],
                                wk_sb[:, k, 128 * p : 128 * (p + 1)],
                                xt_sb[:, k, 512 * sh : 512 * (sh + 1)],
                                start=(k == 0),
                                stop=(k == KT - 1),
                            )

                    def fin():
                        if E_BIAS:
                            nc.scalar.add(dst, cell[0][:, 0, :], b_sb[:, p : p + 1])
                        else:
                            nc.vector.tensor_scalar_add(
                                dst, cell[0][:, 0, :], b_sb[:, p : p + 1]
                            )

                    return alloc, step, fin

                def kmajor(groups):
                    """Emit fills k-step-interleaved so the in-order PE can
                    stream against the DMA arrival of x chunks."""
                    for alloc, _, _ in groups:
                        alloc()
                    for k in range(KT):
                        for _, step, _ in groups:
                            step(k)
                    for _, _, fin in groups:
                        fin()

                def v_fill(wv_sb, t, c0):
                    kmajor([v_steps(wv_sb, t, c0)])

                def kq_fill(wq_sb, xq_sb, p, sh, is_q):
                    kmajor([kq_steps(wq_sb, xq_sb, p, sh, is_q)])

                def emit_av_chunk(u, cd):
                    h = u % 16
                    pt, av = pt_t[u], av_t[u]
                    for sl in range(4):
                        for tt in range(2):
                            skt = 2 * cd + tt
                            nc.tensor.matmul(
                                av[:, sl, :],
                                pt[:, skt, 128 * sl : 128 * (sl + 1)],
                                vsb[:, skt, h, 0:65],
                                start=(sl == 0 and skt == 0),
                                stop=(skt == NSKT - 1),
                                skip_group_check=True,
                            )

                def emit_scores_chunk(u, cd):
                    h, half = u % 16, u // 16
                    p, a = divmod(h, 2)
                    if cd == 0:
                        pt_t[u] = ptp.tile(
                            [128, NSKT, 512], BF16, tag="pt", name=f"pt{u}"
                        )
                        av_t[u] = avp.tile(
                            [128, 4, 65], F32, tag="av", name=f"av{u}"
                        )
                        # HW psum start=True zeroes the whole bank: the very
                        # first AV matmul of the unit is the bank-wide
                        # pre-zero; later writes accumulate with start=False.
                    pt = pt_t[u]
                    sc = scp.tile([128, 2, 512], F32, tag="sc", name=f"sc{u}_{cd}")
                    for tt in range(2):
                        skt = 2 * cd + tt
                        if NODR:
                            nc.tensor.matmul(
                                sc[:, tt, :],
                                kt_sb[
                                    64 * a : 64 * (a + 1),
                                    p,
                                    128 * skt : 128 * (skt + 1),
                                ],
                                qt_sb[
                                    64 * a : 64 * (a + 1),
                                    p,
                                    512 * half : 512 * (half + 1),
                                ],
                                start=True,
                                stop=True,
                            )
                        else:
                            nc.tensor.matmul(
                                sc[:, tt, :],
                                kt_sb[
                                    64 * a : 64 * (a + 1),
                                    p : p + 1,
                                    128 * skt : 128 * (skt + 1),
                                ].broadcast_to([64, 2, 128]),
                                qt_sb[
                                    64 * a : 64 * (a + 1),
                                    p : p + 1,
                                    512 * half : 512 * (half + 1),
                                ].broadcast_to([64, 2, 512]),
                                start=True,
                                stop=True,
                                perf_mode=DR,
                            )
                    c = chunk_ctr[0]
                    chunk_ctr[0] += 1
                    dst = pt[:, 2 * cd : 2 * cd + 2, :]
                    # split exp between ACT and DVE; strictly 1:1 on the last
                    # two units so the tail barrier arrives sooner
                    if (c % 2 == 0) if _fidx[u] >= NU - 2 else (
                        (c * ACT16) % 16 < ACT16
                    ):
                        nc.scalar.activation(dst, sc[:], AF.Exp, scale=ESCALE)
                    else:
                        nc.vector._custom_dve(
                            EXP8,
                            out=dst,
                            in0=sc[:],
                            s0=EXPC2 * (8 * ESCALE) ** 2,
                            s1=EXPC1 * (8 * ESCALE),
                            imm2=EXPC0,
                        )

                def emit_norm(u):
                    h, half = u % 16, u // 16
                    p, odd = divmod(h, 2)
                    av = av_t.pop(u)
                    rc = rcp.tile([128, 4, 1], F32, tag="rc", name=f"rc{u}")
                    nc.vector.reciprocal(rc[:, :, :], av[:, :, 64:65])
                    if odd == 0:
                        # paired layout [128 q, 4 sl, 128=(h%2)*64+d] so the
                        # pair transposes straight into outT's natural layout
                        nt_t[(p, half)] = ntp.tile(
                            [128, 4, 128], BF16, tag="nt", name=f"nt{p}_{half}"
                        )
                    nt = nt_t[(p, half)]
                    for sl in range(4):
                        if E_NORM:
                            nc.scalar.activation(
                                nt[:, sl, 64 * odd : 64 * odd + 64],
                                av[:, sl, 0:64], AF.Copy, scale=rc[:, sl, :],
                            )
                        else:
                            nc.vector.tensor_scalar_mul(
                                nt[:, sl, 64 * odd : 64 * odd + 64],
                                av[:, sl, 0:64], rc[:, sl, :],
                            )
                    if odd == 1:
                        emit_pair_transpose(p, half)

                def emit_pair_transpose(p, half):
                    # [128 q, 128 d-pair] -> [128 d-pair, 128 q] on the HWDGE
                    # xbar: no PE or DVE/ACT cycles spent on the transpose
                    nt = nt_t.pop((p, half))
                    for sl in range(4):
                        qc = 512 * half + 128 * sl
                        nc.sync.dma_start_transpose(
                            outT[:, p, qc : qc + 128], nt[:, sl, :]
                        )

                def emit_y(wo_sb, m):
                    yt = scp.tile([128, 2, 512], F32, tag="sc", name=f"y{m}")
                    ysb = ydp.tile([128, DM], F32, tag="ysb", name=f"ys{m}")
                    for nb in range(2):
                        # bias via a rank-1 matmul (ones^T @ bo2h): the psum
                        # result is final, so the epilogue is a plain copy
                        nc.tensor.matmul(
                            yt[:, nb, :],
                            ones_row[:, 0:128],
                            bo2h[:, 512 * nb : 512 * (nb + 1)],
                            start=True,
                            stop=False,
                            skip_group_check=True,
                        )
                        for k in range(KT):
                            nc.tensor.matmul(
                                yt[:, nb, :],
                                outT[:, k, 128 * m : 128 * (m + 1)],
                                wo_sb[:, k, 512 * nb : 512 * (nb + 1)],
                                start=False,
                                stop=(k == KT - 1),
                            )
                        (nc.scalar.copy if _B("EYCOPY", 1) else nc.vector.tensor_copy)(
                            ysb[:, 512 * nb : 512 * (nb + 1)], yt[:, nb, :]
                        )
                    nc.sync.dma_start(y.ap()[128 * m : 128 * (m + 1), :], ysb[:])

                # ---- flow order: units half-major (all half-0 heads first)
                units = [16 * half + h for half in range(2) for h in range(16)]
                units = [units[i] for i in range(NU)]
                _fidx = {u: i for i, u in enumerate(units)}  # unit -> flow pos
                flow = [(u, cd) for u in units for cd in range(8)]

                # ---- just-in-time fill weave (flow index -> [fns]) --------
                # deadlines (flow chunks): K(p,sh) at 16p+2sh; Q(p,half) at
                # 16p / 128+16p; V(t,0) at ~t//2+LAG; V(t,1) at 64+t//2+LAG.
                extra = {}

                def sched(i, fn):
                    extra.setdefault(i, []).append(fn)

                def run_flow(lo, hi, wv_sb=None, wq_sb=None, xq_sb=None,
                             wo_sb=None):
                    for i in range(lo, hi):
                        if i < len(flow):
                            for fn in extra.get(i, ()):
                                fn()
                            emit_scores_chunk(*flow[i])
                        j = i - LAG
                        if 0 <= j < len(flow):
                            u, cd = flow[j]
                            emit_av_chunk(u, cd)
                            if cd == 7:
                                emit_norm(u)

                with (
                    tc.tile_pool(name="wvp", bufs=1) as wvp,
                    tc.tile_pool(name="wp", bufs=1) as wp,
                ):
                    wv_sb = wvp.tile([128, KT, DM], BF16)
                    wq_sb = wp.tile([128, KT, DM], FP8, tag="wq")
                    xq_sb = wp.tile([128, KT, S if KFP8 else SQ], FP8, tag="xq")

                    # ---- DMA loads, ordered for earliest fill start -------
                    # tiny control tensors first (they gate Pool's in-order
                    # queue and the first bias adds); the ones column of V is
                    # an on-chip memset, not a descriptor-bound DMA.
                    # x streams in column-quarters: K(0,0)/V(0..3) only need
                    # quarter 0, so the PE starts ~5us in instead of ~20us.
                    nc.gpsimd.memset(vsb[:, :, :, 64:65], 1.0)
                    nc.gpsimd.memset(ones_row[:], 1.0)

                    # one large strided DMA per block: HWDGE costs ~625ns per
                    # dma_start regardless of size, so granularity is coarse
                    def ld(dst, src, lo, hi):
                        nc.sync.dma_start(dst[:, :, lo:hi], src.ap()[:, :, lo:hi])

                    ld(xt_sb, xT, 0, 512)          # x quarter 0
                    ld(wk_sb, wk, 0, 256)          # K pairs 0-1
                    ld(wq_sb, wq, 0, 256)
                    nc.sync.dma_start(bk_sb[:], bk.ap())
                    nc.sync.dma_start(bq_sb[:], bq.ap())
                    nc.sync.dma_start(bo2_sb[:], bo2.ap())
                    nc.vector.tensor_copy(bo2h[:], bo2_sb[:])
                    ld(xq_sb, xq8, 0, 512)
                    ld(wv_sb, wv, 0, 512)          # V heads 0-7
                    ld(xq_sb, xq8, 512, 1024)
                    if KFP8:
                        ld(xq_sb, xq8, 1024, 2048)
                    ld(xt_sb, xT, 512, 1024)
                    ld(xt_sb, xT, 1024, 1536)
                    ld(xt_sb, xT, 1536, 2048)
                    ld(wk_sb, wk, 256, 1024)
                    ld(wq_sb, wq, 256, 1024)
                    ld(wv_sb, wv, 512, 1024)

                    K = lambda p, sh: (lambda: kq_fill(wq_sb, xq_sb, p, sh, False))
                    Q = lambda p, sh: (lambda: kq_fill(wq_sb, xq_sb, p, sh, True))
                    V = lambda t, c0: (lambda: v_fill(wv_sb, t, c0))
                    V2 = lambda t, c0: (lambda: kmajor(
                        [v_steps(wv_sb, t, c0), v_steps(wv_sb, t + 1, c0)]
                    ))

                    # K(0,0)+Q(0,0) k-interleaved gate flow 0; V fills in
                    # k-major pairs stream with the x quarters
                    sched(0, lambda: kmajor([
                        kq_steps(wq_sb, xq_sb, 0, 0, False),
                        kq_steps(wq_sb, xq_sb, 0, 0, True),
                    ]))
                    for i in range(8):    # V(2i, 2i+1) pairs at flows 1..8
                        sched(1 + i, V2(2 * i, 0))
                    sched(1, K(0, 1)); sched(2, K(0, 2)); sched(3, K(0, 3))
                    sched(5, Q(1, 0))
                    sched(6, K(1, 0)); sched(8, K(1, 1))
                    sched(10, K(1, 2)); sched(12, K(1, 3))
                    for p in range(2, 8):
                        sched(16 * p - 6, Q(p, 0))
                        for sh in range(4):
                            sched(16 * p + 2 * sh - 7, K(p, sh))
                    for t in range(16):   # V c0=1 at flows 40..55
                        sched(40 + t, V(t, 1))
                    for p in range(8):    # Q half-1 at flows 56..63
                        sched(56 + p, Q(p, 1))

                    run_flow(0, 64)

                # wv/wq/xq freed; wo loads into the released space
                with tc.tile_pool(name="wop", bufs=1) as wop:
                    wo_sb = wop.tile([128, KT, DM], BF16)
                    nc.sync.dma_start(wo_sb[:], wo.ap())

                    # first-half y blocks interleave into the half-1 flow
                    for m in range(4):
                        sched(_B("YFLOW", 142) + 6 * m,
                              (lambda m=m: emit_y(wo_sb, m)))

                    run_flow(64, len(flow) + LAG)
                    for m in range(4, 8):
                        emit_y(wo_sb, m)

    nc.compile()
    return nc


def prep_inputs(x, Wq, bq, Wk, bk, Wv, bv, Wo, bo):
    """Host-side sharding + layout permutations (numpy only)."""
    import ml_dtypes

    bf16 = ml_dtypes.bfloat16
    x = np.asarray(x, np.float32)
    Wq = np.asarray(Wq, np.float32)
    Wk = np.asarray(Wk, np.float32)
    Wv = np.asarray(Wv, np.float32)
    Wo = np.asarray(Wo, np.float32)
    bq = np.asarray(bq, np.float32)
    bk = np.asarray(bk, np.float32)
    bv = np.asarray(bv, np.float32)
    bo = np.asarray(bo, np.float32)

    def to3(Wm):  # [M=1024 rows, 1024 cols] -> [128, 8, 1024]
        return np.ascontiguousarray(Wm.reshape(KT, 128, DM).transpose(1, 0, 2))

    def natural(W):  # [H, M, hd] -> [M, (h, d)]
        return np.ascontiguousarray(W.transpose(1, 0, 2).reshape(DM, DM))

    def fold_bias(b):
        # [H, hd] -> [128 part=(h%2)*64+d, 8 col=pair]
        return np.ascontiguousarray(b.reshape(KT, 128).T)

    import ml_dtypes as _mld
    fp8 = _mld.float8_e4m3
    kfp8 = _B("KFP8", 0)
    wk_h = to3(natural(Wk)).astype(fp8 if kfp8 else bf16)
    wq_h = to3(natural(Wq)).astype(fp8)
    wv_h = to3(natural(Wv)).astype(bf16)
    # wo rows permuted: row (c, p) = Wo^T[dm] with dm = (2c + p//64)*64 + p%64
    WoT = np.ascontiguousarray(Wo.T)              # [dm, n]
    cidx = np.arange(KT)[None, :]
    pidx = np.arange(128)[:, None]
    dmidx = (2 * cidx + pidx // 64) * 64 + (pidx % 64)   # [128, 8]
    wo_h = np.ascontiguousarray(WoT[dmidx.transpose(), :].reshape(KT, 128, DM)
                                .transpose(1, 0, 2)).astype(bf16)
    bo2 = (bo + Wo @ bv.reshape(-1)).reshape(1, DM).astype(np.float32)

    shared = {
        "wk": wk_h,
        "wq": wq_h,
        "wv": wv_h,
        "wo": wo_h,
        "bk": fold_bias(bk),
        "bq": fold_bias(bq),
        "bo2": bo2,
    }
    in_maps = []
    for core in range(N_CORES):
        b, half = divmod(core, 2)
        xt = x[b].T
        if half == 1:
            xt = np.concatenate([xt[:, SQ:], xt[:, :SQ]], axis=1)
        xt3 = np.ascontiguousarray(
            xt.reshape(KT, 128, S).transpose(1, 0, 2)
        ).astype(bf16)
        ncol = S if kfp8 else SQ
        xq3 = np.ascontiguousarray(xt3[:, :, 0:ncol].astype(np.float32)).astype(fp8)
        in_maps.append({"xT": xt3, "xq8": xq3, **shared})
    return in_maps


def assemble_output(results):
    y = np.empty((B, S, DM), dtype=np.float32)
    for core in range(N_CORES):
        b, half = divmod(core, 2)
        y[b, half * SQ : (half + 1) * SQ, :] = results[core]["y"]
    return y


def _get_runner():
    """Build the program + jitted 8-core executor once; reuse across calls."""
    if "runner" in _CACHE:
        return _CACHE["runner"]

    import jax
    import concourse.mybir as mb
    from concourse import bass2jax
    from jax.sharding import Mesh, PartitionSpec
    from jax.experimental.shard_map import shard_map

    nc = build_program()
    _CACHE["nc"] = nc
    bass2jax.install_neuronx_cc_hook()

    partition_name = (
        nc.partition_id_tensor.name if nc.partition_id_tensor is not None else None
    )
    in_names, out_names, out_avals = [], [], []
    for alloc in nc.m.functions[0].allocations:
        if not isinstance(alloc, mb.MemoryLocationSet):
            continue
        name = alloc.memorylocations[0].name
        if alloc.kind == "ExternalInput":
            if name != partition_name:
                in_names.append(name)
        elif alloc.kind == "ExternalOutput":
            out_names.append(name)
            out_avals.append(
                jax.core.ShapedArray(tuple(alloc.tensor_shape), mb.dt.np(alloc.dtype))
            )
    n_params = len(in_names)
    n_outs = len(out_avals)
    all_in_names = in_names + out_names
    if partition_name is not None:
        all_in_names = all_in_names + [partition_name]

    def _body(*args):
        operands = list(args)
        if partition_name is not None:
            operands.append(bass2jax.partition_id_tensor())
        outs = bass2jax._bass_exec_p.bind(
            *operands,
            out_avals=tuple(out_avals),
            in_names=tuple(all_in_names),
            out_names=tuple(out_names),
            lowering_input_output_aliases=(),
            sim_require_finite=True,
            sim_require_nnan=True,
            nc=nc,
        )
        return tuple(outs)

    devices = jax.devices()[:N_CORES]
    mesh = Mesh(np.asarray(devices), ("core",))
    donate = tuple(range(n_params, n_params + n_outs))
    sharded = jax.jit(
        shard_map(
            _body,
            mesh=mesh,
            in_specs=(PartitionSpec("core"),) * (n_params + n_outs),
            out_specs=(PartitionSpec("core"),) * n_outs,
            check_rep=False,
        ),
        donate_argnums=donate,
        keep_unused=True,
    )

    import hashlib

    from jax.sharding import NamedSharding

    sharding = NamedSharding(mesh, PartitionSpec("core"))
    dev_cache: dict = {}

    import jax.numpy as jnp

    zeros_fns = [
        jax.jit(
            (lambda shape, dtype: (lambda: jnp.zeros(shape, dtype)))(
                (N_CORES * a.shape[0], *a.shape[1:]), a.dtype
            ),
            out_shardings=sharding,
        )
        for a in out_avals
    ]

    def _dev_input(nm, in_maps):
        arrs = [np.asarray(m[nm]) for m in in_maps]
        h = hashlib.blake2b(digest_size=16)
        for a in arrs:
            h.update(a.tobytes())
        key = (nm, h.hexdigest())
        if key not in dev_cache:
            if len(dev_cache) > 64:
                dev_cache.clear()
            dev_cache[key] = jax.device_put(
                np.concatenate(arrs, axis=0), sharding
            )
        return dev_cache[key]

    def run(in_maps):
        concat_in = [_dev_input(nm, in_maps) for nm in in_names]
        concat_zeros = [zf() for zf in zeros_fns]
        out_arrs = sharded(*concat_in, *concat_zeros)
        return [
            {
                nm: np.asarray(out_arrs[i]).reshape(N_CORES, *out_avals[i].shape)[c]
                for i, nm in enumerate(out_names)
            }
            for c in range(N_CORES)
        ]

    _CACHE["runner"] = run
    return run


def kernel(**inputs):
    run = _get_runner()
    in_maps = prep_inputs(**inputs)
    return assemble_output(run(in_maps))


# revision 35
# speedup vs baseline: 1.1801x; 1.1801x over previous
"""Multi-head attention kernel for Trainium2, 8 NeuronCores — v3.

Sharding: data-parallel over (batch, query-half): core i handles batch i//2
and query rows (i%2)*1024 ... +1024 (no collectives; K/V projection duplicated
between the 2 cores of a batch).

Per-core dataflow, all activation tiles SBUF-resident (no DRAM scratch):
  xT   bf16 [128, 8k, 2048]       (own query-half columns first)
  K^T, Q^T: bf16 matmul + bias, requantized fp8e4, natural pair layout
        [128=(h%2)*64+d, pair, s] — the only fp8 tensors in the pipeline
  V    bf16 [128 sk, 16 t, 16 h, 66] with a ones column per head (col 64)
  scores^T[sk, sq]: fp8 DoubleRow matmul, both operands broadcast_to a
        stride-0 slot dim so the PE computes 2*K^T Q at 0.5 cycles/row;
        the factor 2 folds into the exp scale (0.0625)
  P^T  = exp(scores/16): split between ACT (Exp) and a custom DVE op
        (EXP8_MHA polynomial) at a tunable ratio (default 9:7 per 16 chunks)
  AV:  out[sq, 4 sl, 65] = P^T-tile.T @ [V|1], N=65 bf16; the unit's first
        AV matmul start=True pre-zeroes the whole psum bank
  norm: reciprocal (DVE) + per-sl scalar mul (Pool)
  transpose back to out^T via PE identity matmul; psum->SBUF copy on Pool
  y    = outT.T @ Wo^T + bo' (bo' = bo + Wo@bv host-folded), f32 out

v3 scheduling: units run half-major (all query-half-0 heads first, then
half-1) so the first half of the output projection interleaves with the
second half of the score/AV flow instead of serializing at the end. All
non-exp elementwise work (bias adds, V copies, norm muls, outT copies,
y bias) runs on the idle Pool/GPSIMD engine so ACT+DVE split exp evenly.
DMA loads are ordered x+first-half weights first so projection fills start
~immediately; wo loads into SBUF space freed by wv/wq/xq mid-flow.
"""

import os

os.environ.setdefault("MYCRO_LOCAL_CACHE", "1")

import numpy as np

_B = lambda k, d: int(os.environ.get(k, d))

try:
    import concourse.bass as bass
except ImportError:  # pragma: no cover
    import sys

    for p in ("/opt/trn_rl_repo", "/root/.axon_site/_ro/trn_rl_repo"):
        if os.path.isdir(p) and p not in sys.path:
            sys.path.insert(0, p)
    import concourse.bass as bass

import concourse.mybir as mybir
import concourse.tile as tile
from concourse import bacc, bass_utils

BF16 = mybir.dt.bfloat16
F32 = mybir.dt.float32
FP8 = mybir.dt.float8e4
AF = mybir.ActivationFunctionType
DR = mybir.MatmulPerfMode.DoubleRow

B = 4
S = 2048
DM = 1024
H = 16
HD = 64
KT = 8          # d_model contraction chunks of 128
NG = 4          # head groups of 4
NSKT = 16       # sk tiles of 128
SQ = 1024       # query rows per core
NU = 32         # units = (head, sq-half of 512)
N_CORES = 8

# quadratic p(s) ~= exp(s/64); P = p(s)^8 = exp(s/8). Minimax on |s/64|<=0.3
EXPC2, EXPC1, EXPC0 = 1.215639159300168e-04, 1.5754060152766296e-02, 1.0001349756688513

_CACHE: dict = {}


def _register_exp8():
    """Register the custom DVE op EXP8_MHA (documented dve_ops extension
    point, done at runtime so kernel.py stays self-contained)."""
    import concourse.dve_ops as dve_ops
    from concourse.dve_spec import Spec, Src0, C0, C1, C2, sq as dsq
    from concourse.dve_spec import lower as dve_lower
    from concourse.dve_uop import DveOpSpec

    name = "EXP8_MHA"
    if name in dve_ops._SUB_OPCODE_FOR_NAME:
        return dve_ops._BY_NAME_EXP8

    def _ref(in0, in1, s0, s1, imm2):
        x = np.asarray(in0, np.float32)
        p = ((x * np.float32(s0) + np.float32(s1)) * x + np.float32(imm2)).astype(
            np.float32
        )
        p = (p * p).astype(np.float32)
        p = (p * p).astype(np.float32)
        p = (p * p).astype(np.float32)
        return p

    body = dsq(dsq(dsq((Src0 * C0 + C1) * Src0 + C2)))
    spec = Spec(body=body, reference=_ref)
    row = dve_ops._CUSTOM_DVE_ROW_BASE + len(dve_ops.OPS)
    shas = {}
    for ver in ("v3", "v4"):
        uops = dve_lower(spec, ver=ver)
        shas[ver] = DveOpSpec(name=name, opcode=row, uops=uops, rd1_en=False).sha(ver)
    op = dve_ops.DveOp(name, spec, subdim=False, uops_sha=shas)
    dve_ops.OPS.append(op)
    dve_ops.CUSTOM_DVE_SPECS[name] = spec
    dve_ops._SUB_OPCODE_FOR_NAME[name] = row
    dve_ops._BY_NAME_EXP8 = op
    return op


def build_program():
    EXP8 = _register_exp8()
    nc = bacc.Bacc("TRN2", target_bir_lowering=False, debug=False)

    KFP8 = _B("KFP8", 0)
    QBF16 = _B("QBF16", 0)
    xT = nc.dram_tensor("xT", [128, KT, S], BF16, kind="ExternalInput")
    wk = nc.dram_tensor("wk", [128, KT, DM], FP8 if KFP8 else BF16,
                        kind="ExternalInput")
    xq8 = nc.dram_tensor("xq8", [128, KT, S if KFP8 else SQ], FP8,
                         kind="ExternalInput")
    wq = nc.dram_tensor("wq", [128, KT, DM], BF16 if QBF16 else FP8,
                        kind="ExternalInput")
    wv = nc.dram_tensor("wv", [128, KT, DM], BF16, kind="ExternalInput")
    wo = nc.dram_tensor("wo", [128, KT, DM], BF16, kind="ExternalInput")
    bk = nc.dram_tensor("bk", [128, KT], F32, kind="ExternalInput")
    bq = nc.dram_tensor("bq", [128, KT], F32, kind="ExternalInput")
    bo2 = nc.dram_tensor("bo2", [1, DM], F32, kind="ExternalInput")
    y = nc.dram_tensor("y", [SQ, DM], F32, kind="ExternalOutput")

    NODR = _B("NODR", 0)
    ESCALE = 0.125 if NODR else 0.0625
    ACT16 = _B("ACT16", 9)   # of every 16 exp chunks, this many go to ACT
    LAG = _B("AVLAG", 10)
    # engine choice per elementwise-op class: 0 = DVE, 1 = ACT
    # (GPSIMD/Pool cannot touch PSUM, so all of these are ACT-or-DVE)
    E_BIAS = _B("EBIAS", 0)
    E_VCOPY = _B("EVCOPY", 1)
    E_NORM = _B("ENORM", 0)

    with tile.TileContext(nc) as tc:
        with tc.tile_pool(name="pers", bufs=1) as pers:
            bo2_sb = pers.tile([1, DM], F32)
            bo2h = pers.tile([1, DM], BF16)
            ones_row = pers.tile([1, 128], BF16)
            bk_sb = pers.tile([128, KT], F32)
            bq_sb = pers.tile([128, KT], F32)
            # natural pair layout: pair p = heads (2p, 2p+1); head h on
            # partitions [64*(h%2), 64*(h%2)+64)
            kt_sb = pers.tile([128, 8, S], FP8)          # 16 KiB/part
            qt_sb = pers.tile([128, 8, SQ], FP8)         # 8 KiB/part
            vsb = pers.tile([128, NSKT, H, 66], BF16)    # 32.5 KiB/part
            outT = pers.tile([128, KT, SQ], BF16)        # 16 KiB/part
            wk_sb = pers.tile([128, KT, DM], FP8 if KFP8 else BF16)

            with (
                tc.tile_pool(name="scp", bufs=_B("SCB", 3), space="PSUM") as scp,
                tc.tile_pool(name="ptp", bufs=_B("PTB", 2)) as ptp,
                tc.tile_pool(name="ntp", bufs=2) as ntp,
                tc.tile_pool(name="rcp", bufs=2) as rcp,
                tc.tile_pool(name="xp", bufs=1) as xp,
                tc.tile_pool(name="avp", bufs=2, space="PSUM") as avp,
                tc.tile_pool(name="ydp", bufs=3) as ydp,
            ):
                xt_sb = xp.tile([128, KT, S], BF16)

                pt_t, av_t, nt_t = {}, {}, {}
                chunk_ctr = [0]

                def v_steps(wv_sb, t, c0):
                    """(psum-alloc, [per-k matmul step], epilogue) for a V fill."""
                    cell = [None]

                    def alloc():
                        cell[0] = scp.tile(
                            [128, 2, 512], F32, tag="sc", name=f"v{t}_{c0}"
                        )

                    def step(k):
                        nc.tensor.matmul(
                            cell[0][:, 0, :],
                            xt_sb[:, k, 128 * t : 128 * (t + 1)],
                            wv_sb[:, k, 512 * c0 : 512 * (c0 + 1)],
                            start=(k == 0),
                            stop=(k == KT - 1),
                        )

                    def fin():
                        if E_VCOPY:
                            nc.scalar.copy(
                                vsb[:, t, 8 * c0 : 8 * (c0 + 1), 0:64],
                                cell[0][:, 0, :],
                            )
                        else:
                            nc.vector.tensor_copy(
                                vsb[:, t, 8 * c0 : 8 * (c0 + 1), 0:64],
                                cell[0][:, 0, :],
                            )

                    return alloc, step, fin

                def kq_steps(wq_sb, xq_sb, p, sh, is_q):
                    b_sb = bq_sb if is_q else bk_sb
                    dst = (qt_sb if is_q else kt_sb)[:, p, 512 * sh : 512 * (sh + 1)]
                    cell = [None]

                    def alloc():
                        cell[0] = scp.tile(
                            [128, 2, 512], F32, tag="sc",
                            name=f"{'q' if is_q else 'k'}{p}_{sh}",
                        )

                    def step(k):
                        if is_q or KFP8:
                            # fp8 DoubleRow over k-chunk pairs; emit at odd k
                            if k % 2 == 0:
                                return
                            kp = k // 2
                            w = wq_sb if is_q else wk_sb
                            nc.tensor.matmul(
                                cell[0][:, 0, :],
                                w[:, 2 * kp : 2 * kp + 2, 128 * p : 128 * (p + 1)],
                                xq_sb[:, 2 * kp : 2 * kp + 2, 512 * sh : 512 * (sh + 1)],
                                start=(kp == 0),
                                stop=(kp == KT // 2 - 1),
                                perf_mode=DR,
                            )
                        else:
                            nc.tensor.matmul(
                                cell[0][:, 0, :],
                                wk_sb[:, k, 128 * p : 128 * (p + 1)],
                                xt_sb[:, k, 512 * sh : 512 * (sh + 1)],
                                start=(k == 0),
                                stop=(k == KT - 1),
                            )

                    def fin():
                        if E_BIAS:
                            nc.scalar.add(dst, cell[0][:, 0, :], b_sb[:, p : p + 1])
                        else:
                            nc.vector.tensor_scalar_add(
                                dst, cell[0][:, 0, :], b_sb[:, p : p + 1]
                            )

                    return alloc, step, fin

                def kmajor(groups):
                    """Emit fills k-step-interleaved so the in-order PE can
                    stream against the DMA arrival of x chunks."""
                    for alloc, _, _ in groups:
                        alloc()
                    for k in range(KT):
                        for _, step, _ in groups:
                            step(k)
                    for _, _, fin in groups:
                        fin()

                def v_fill(wv_sb, t, c0):
                    kmajor([v_steps(wv_sb, t, c0)])

                def kq_fill(wq_sb, xq_sb, p, sh, is_q):
                    kmajor([kq_steps(wq_sb, xq_sb, p, sh, is_q)])

                def emit_av_chunk(u, cd):
                    h = u % 16
                    pt, av = pt_t[u], av_t[u]
                    for sl in range(4):
                        for tt in range(2):
                            skt = 2 * cd + tt
                            nc.tensor.matmul(
                                av[:, sl, :],
                                pt[:, skt, 128 * sl : 128 * (sl + 1)],
                                vsb[:, skt, h, 0:65],
                                start=(sl == 0 and skt == 0),
                                stop=(skt == NSKT - 1),
                                skip_group_check=True,
                            )

                def emit_scores_chunk(u, cd):
                    h, half = u % 16, u // 16
                    p, a = divmod(h, 2)
                    if cd == 0:
                        pt_t[u] = ptp.tile(
                            [128, NSKT, 512], BF16, tag="pt", name=f"pt{u}"
                        )
                        av_t[u] = avp.tile(
                            [128, 4, 65], F32, tag="av", name=f"av{u}"
                        )
                        # HW psum start=True zeroes the whole bank: the very
                        # first AV matmul of the unit is the bank-wide
                        # pre-zero; later writes accumulate with start=False.
                    pt = pt_t[u]
                    sc = scp.tile([128, 2, 512], F32, tag="sc", name=f"sc{u}_{cd}")
                    for tt in range(2):
                        skt = 2 * cd + tt
                        if NODR:
                            nc.tensor.matmul(
                                sc[:, tt, :],
                                kt_sb[
                                    64 * a : 64 * (a + 1),
                                    p,
                                    128 * skt : 128 * (skt + 1),
                                ],
                                qt_sb[
                                    64 * a : 64 * (a + 1),
                                    p,
                                    512 * half : 512 * (half + 1),
                                ],
                                start=True,
                                stop=True,
                            )
                        else:
                            nc.tensor.matmul(
                                sc[:, tt, :],
                                kt_sb[
                                    64 * a : 64 * (a + 1),
                                    p : p + 1,
                                    128 * skt : 128 * (skt + 1),
                                ].broadcast_to([64, 2, 128]),
                                qt_sb[
                                    64 * a : 64 * (a + 1),
                                    p : p + 1,
                                    512 * half : 512 * (half + 1),
                                ].broadcast_to([64, 2, 512]),
                                start=True,
                                stop=True,
                                perf_mode=DR,
                            )
                    c = chunk_ctr[0]
                    chunk_ctr[0] += 1
                    dst = pt[:, 2 * cd : 2 * cd + 2, :]
                    # split exp between ACT and DVE; strictly 1:1 on the last
                    # two units so the tail barrier arrives sooner
                    if (c % 2 == 0) if _fidx[u] >= NU - 2 else (
                        (c * ACT16) % 16 < ACT16
                    ):
                        nc.scalar.activation(dst, sc[:], AF.Exp, scale=ESCALE)
                    else:
                        nc.vector._custom_dve(
                            EXP8,
                            out=dst,
                            in0=sc[:],
                            s0=EXPC2 * (8 * ESCALE) ** 2,
                            s1=EXPC1 * (8 * ESCALE),
                            imm2=EXPC0,
                        )

                def emit_norm(u):
                    h, half = u % 16, u // 16
                    p, odd = divmod(h, 2)
                    av = av_t.pop(u)
                    rc = rcp.tile([128, 4, 1], F32, tag="rc", name=f"rc{u}")
                    nc.vector.reciprocal(rc[:, :, :], av[:, :, 64:65])
                    if odd == 0:
                        # paired layout [128 q, 4 sl, 128=(h%2)*64+d] so the
                        # pair transposes straight into outT's natural layout
                        nt_t[(p, half)] = ntp.tile(
                            [128, 4, 128], BF16, tag="nt", name=f"nt{p}_{half}"
                        )
                    nt = nt_t[(p, half)]
                    for sl in range(4):
                        if E_NORM:
                            nc.scalar.activation(
                                nt[:, sl, 64 * odd : 64 * odd + 64],
                                av[:, sl, 0:64], AF.Copy, scale=rc[:, sl, :],
                            )
                        else:
                            nc.vector.tensor_scalar_mul(
                                nt[:, sl, 64 * odd : 64 * odd + 64],
                                av[:, sl, 0:64], rc[:, sl, :],
                            )
                    if odd == 1:
                        emit_pair_transpose(p, half)

                def emit_pair_transpose(p, half):
                    # [128 q, 128 d-pair] -> [128 d-pair, 128 q] on the HWDGE
                    # xbar: no PE or DVE/ACT cycles spent on the transpose
                    nt = nt_t.pop((p, half))
                    for sl in range(4):
                        qc = 512 * half + 128 * sl
                        nc.sync.dma_start_transpose(
                            outT[:, p, qc : qc + 128], nt[:, sl, :]
                        )

                def emit_y(wo_sb, m):
                    yt = scp.tile([128, 2, 512], F32, tag="sc", name=f"y{m}")
                    ysb = ydp.tile([128, DM], F32, tag="ysb", name=f"ys{m}")
                    for nb in range(2):
                        # bias via a rank-1 matmul (ones^T @ bo2h): the psum
                        # result is final, so the epilogue is a plain copy
                        nc.tensor.matmul(
                            yt[:, nb, :],
                            ones_row[:, 0:128],
                            bo2h[:, 512 * nb : 512 * (nb + 1)],
                            start=True,
                            stop=False,
                            skip_group_check=True,
                        )
                        for k in range(KT):
                            nc.tensor.matmul(
                                yt[:, nb, :],
                                outT[:, k, 128 * m : 128 * (m + 1)],
                                wo_sb[:, k, 512 * nb : 512 * (nb + 1)],
                                start=False,
                                stop=(k == KT - 1),
                            )
                        (nc.scalar.copy if _B("EYCOPY", 1) else nc.vector.tensor_copy)(
                            ysb[:, 512 * nb : 512 * (nb + 1)], yt[:, nb, :]
                        )
                    nc.sync.dma_start(y.ap()[128 * m : 128 * (m + 1), :], ysb[:])

                # ---- flow order: units half-major (all half-0 heads first)
                units = [16 * half + h for half in range(2) for h in range(16)]
                units = [units[i] for i in range(NU)]
                _fidx = {u: i for i, u in enumerate(units)}  # unit -> flow pos
                flow = [(u, cd) for u in units for cd in range(8)]

                # ---- just-in-time fill weave (flow index -> [fns]) --------
                # deadlines (flow chunks): K(p,sh) at 16p+2sh; Q(p,half) at
                # 16p / 128+16p; V(t,0) at ~t//2+LAG; V(t,1) at 64+t//2+LAG.
                extra = {}

                def sched(i, fn):
                    extra.setdefault(i, []).append(fn)

                def run_flow(lo, hi, wv_sb=None, wq_sb=None, xq_sb=None,
                             wo_sb=None):
                    for i in range(lo, hi):
                        if i < len(flow):
                            for fn in extra.get(i, ()):
                                fn()
                            emit_scores_chunk(*flow[i])
                        j = i - LAG
                        if 0 <= j < len(flow):
                            u, cd = flow[j]
                            emit_av_chunk(u, cd)
                            if cd == 7:
                                emit_norm(u)

                with (
                    tc.tile_pool(name="wvp", bufs=1) as wvp,
                    tc.tile_pool(name="wp", bufs=1) as wp,
                ):
                    wv_sb = wvp.tile([128, KT, DM], BF16)
                    wq_sb = wp.tile([128, KT, DM], FP8, tag="wq")
                    xq_sb = wp.tile([128, KT, S if KFP8 else SQ], FP8, tag="xq")

                    # ---- DMA loads, ordered for earliest fill start -------
                    # tiny control tensors first (they gate Pool's in-order
                    # queue and the first bias adds); the ones column of V is
                    # an on-chip memset, not a descriptor-bound DMA.
                    # x streams in column-quarters: K(0,0)/V(0..3) only need
                    # quarter 0, so the PE starts ~5us in instead of ~20us.
                    nc.gpsimd.memset(vsb[:, :, :, 64:65], 1.0)
                    nc.gpsimd.memset(ones_row[:], 1.0)

                    # one large strided DMA per block: HWDGE costs ~625ns per
                    # dma_start regardless of size, so granularity is coarse
                    def ld(dst, src, lo, hi):
                        nc.sync.dma_start(dst[:, :, lo:hi], src.ap()[:, :, lo:hi])

                    ld(xt_sb, xT, 0, 512)          # x quarter 0
                    ld(wk_sb, wk, 0, 256)          # K pairs 0-1
                    ld(wq_sb, wq, 0, 256)
                    nc.sync.dma_start(bk_sb[:], bk.ap())
                    nc.sync.dma_start(bq_sb[:], bq.ap())
                    nc.sync.dma_start(bo2_sb[:], bo2.ap())
                    nc.vector.tensor_copy(bo2h[:], bo2_sb[:])
                    ld(xq_sb, xq8, 0, 512)
                    ld(wv_sb, wv, 0, 512)          # V heads 0-7
                    ld(xq_sb, xq8, 512, 1024)
                    if KFP8:
                        ld(xq_sb, xq8, 1024, 2048)
                    ld(xt_sb, xT, 512, 1024)
                    ld(xt_sb, xT, 1024, 1536)
                    ld(xt_sb, xT, 1536, 2048)
                    ld(wk_sb, wk, 256, 1024)
                    ld(wq_sb, wq, 256, 1024)
                    ld(wv_sb, wv, 512, 1024)

                    K = lambda p, sh: (lambda: kq_fill(wq_sb, xq_sb, p, sh, False))
                    Q = lambda p, sh: (lambda: kq_fill(wq_sb, xq_sb, p, sh, True))
                    V = lambda t, c0: (lambda: v_fill(wv_sb, t, c0))
                    V2 = lambda t, c0: (lambda: kmajor(
                        [v_steps(wv_sb, t, c0), v_steps(wv_sb, t + 1, c0)]
                    ))

                    # K(0,0)+Q(0,0) k-interleaved gate flow 0; V fills in
                    # k-major pairs stream with the x quarters
                    sched(0, lambda: kmajor([
                        kq_steps(wq_sb, xq_sb, 0, 0, False),
                        kq_steps(wq_sb, xq_sb, 0, 0, True),
                    ]))
                    for i in range(8):    # V(2i, 2i+1) pairs at flows 1..8
                        sched(1 + i, V2(2 * i, 0))
                    sched(1, K(0, 1)); sched(2, K(0, 2)); sched(3, K(0, 3))
                    sched(5, Q(1, 0))
                    sched(6, K(1, 0)); sched(8, K(1, 1))
                    sched(10, K(1, 2)); sched(12, K(1, 3))
                    for p in range(2, 8):
                        sched(16 * p - 6, Q(p, 0))
                        for sh in range(4):
                            sched(16 * p + 2 * sh - 7, K(p, sh))
                    VF1 = _B("VF1", 56)   # V c0=1: deadline is flow 64+t//2
                    for t in range(16):
                        sched(VF1 + t, V(t, 1))
                    for p in range(8):    # Q half-1 before the wq pool closes
                        sched(40 + p, Q(p, 1))

                    run_flow(0, _B("SEGB", 80))

                # wv/wq/xq freed; wo loads into the released space
                with tc.tile_pool(name="wop", bufs=1) as wop:
                    wo_sb = wop.tile([128, KT, DM], BF16)
                    nc.sync.dma_start(wo_sb[:], wo.ap())

                    # first-half y blocks interleave into the half-1 flow
                    for m in range(4):
                        sched(_B("YFLOW", 142) + 6 * m,
                              (lambda m=m: emit_y(wo_sb, m)))

                    run_flow(_B("SEGB", 80), len(flow) + LAG)
                    for m in range(4, 8):
                        emit_y(wo_sb, m)

    nc.compile()
    return nc


def prep_inputs(x, Wq, bq, Wk, bk, Wv, bv, Wo, bo):
    """Host-side sharding + layout permutations (numpy only)."""
    import ml_dtypes

    bf16 = ml_dtypes.bfloat16
    x = np.asarray(x, np.float32)
    Wq = np.asarray(Wq, np.float32)
    Wk = np.asarray(Wk, np.float32)
    Wv = np.asarray(Wv, np.float32)
    Wo = np.asarray(Wo, np.float32)
    bq = np.asarray(bq, np.float32)
    bk = np.asarray(bk, np.float32)
    bv = np.asarray(bv, np.float32)
    bo = np.asarray(bo, np.float32)

    def to3(Wm):  # [M=1024 rows, 1024 cols] -> [128, 8, 1024]
        return np.ascontiguousarray(Wm.reshape(KT, 128, DM).transpose(1, 0, 2))

    def natural(W):  # [H, M, hd] -> [M, (h, d)]
        return np.ascontiguousarray(W.transpose(1, 0, 2).reshape(DM, DM))

    def fold_bias(b):
        # [H, hd] -> [128 part=(h%2)*64+d, 8 col=pair]
        return np.ascontiguousarray(b.reshape(KT, 128).T)

    import ml_dtypes as _mld
    fp8 = _mld.float8_e4m3
    kfp8 = _B("KFP8", 0)
    wk_h = to3(natural(Wk)).astype(fp8 if kfp8 else bf16)
    wq_h = to3(natural(Wq)).astype(fp8)
    wv_h = to3(natural(Wv)).astype(bf16)
    # wo rows permuted: row (c, p) = Wo^T[dm] with dm = (2c + p//64)*64 + p%64
    WoT = np.ascontiguousarray(Wo.T)              # [dm, n]
    cidx = np.arange(KT)[None, :]
    pidx = np.arange(128)[:, None]
    dmidx = (2 * cidx + pidx // 64) * 64 + (pidx % 64)   # [128, 8]
    wo_h = np.ascontiguousarray(WoT[dmidx.transpose(), :].reshape(KT, 128, DM)
                                .transpose(1, 0, 2)).astype(bf16)
    bo2 = (bo + Wo @ bv.reshape(-1)).reshape(1, DM).astype(np.float32)

    shared = {
        "wk": wk_h,
        "wq": wq_h,
        "wv": wv_h,
        "wo": wo_h,
        "bk": fold_bias(bk),
        "bq": fold_bias(bq),
        "bo2": bo2,
    }
    in_maps = []
    for core in range(N_CORES):
        b, half = divmod(core, 2)
        xt = x[b].T
        if half == 1:
            xt = np.concatenate([xt[:, SQ:], xt[:, :SQ]], axis=1)
        xt3 = np.ascontiguousarray(
            xt.reshape(KT, 128, S).transpose(1, 0, 2)
        ).astype(bf16)
        ncol = S if kfp8 else SQ
        xq3 = np.ascontiguousarray(xt3[:, :, 0:ncol].astype(np.float32)).astype(fp8)
        in_maps.append({"xT": xt3, "xq8": xq3, **shared})
    return in_maps


def assemble_output(results):
    y = np.empty((B, S, DM), dtype=np.float32)
    for core in range(N_CORES):
        b, half = divmod(core, 2)
        y[b, half * SQ : (half + 1) * SQ, :] = results[core]["y"]
    return y


def _get_runner():
    """Build the program + jitted 8-core executor once; reuse across calls."""
    if "runner" in _CACHE:
        return _CACHE["runner"]

    import jax
    import concourse.mybir as mb
    from concourse import bass2jax
    from jax.sharding import Mesh, PartitionSpec
    from jax.experimental.shard_map import shard_map

    nc = build_program()
    _CACHE["nc"] = nc
    bass2jax.install_neuronx_cc_hook()

    partition_name = (
        nc.partition_id_tensor.name if nc.partition_id_tensor is not None else None
    )
    in_names, out_names, out_avals = [], [], []
    for alloc in nc.m.functions[0].allocations:
        if not isinstance(alloc, mb.MemoryLocationSet):
            continue
        name = alloc.memorylocations[0].name
        if alloc.kind == "ExternalInput":
            if name != partition_name:
                in_names.append(name)
        elif alloc.kind == "ExternalOutput":
            out_names.append(name)
            out_avals.append(
                jax.core.ShapedArray(tuple(alloc.tensor_shape), mb.dt.np(alloc.dtype))
            )
    n_params = len(in_names)
    n_outs = len(out_avals)
    all_in_names = in_names + out_names
    if partition_name is not None:
        all_in_names = all_in_names + [partition_name]

    def _body(*args):
        operands = list(args)
        if partition_name is not None:
            operands.append(bass2jax.partition_id_tensor())
        outs = bass2jax._bass_exec_p.bind(
            *operands,
            out_avals=tuple(out_avals),
            in_names=tuple(all_in_names),
            out_names=tuple(out_names),
            lowering_input_output_aliases=(),
            sim_require_finite=True,
            sim_require_nnan=True,
            nc=nc,
        )
        return tuple(outs)

    devices = jax.devices()[:N_CORES]
    mesh = Mesh(np.asarray(devices), ("core",))
    donate = tuple(range(n_params, n_params + n_outs))
    sharded = jax.jit(
        shard_map(
            _body,
            mesh=mesh,
            in_specs=(PartitionSpec("core"),) * (n_params + n_outs),
            out_specs=(PartitionSpec("core"),) * n_outs,
            check_rep=False,
        ),
        donate_argnums=donate,
        keep_unused=True,
    )

    import hashlib

    from jax.sharding import NamedSharding

    sharding = NamedSharding(mesh, PartitionSpec("core"))
    dev_cache: dict = {}

    import jax.numpy as jnp

    zeros_fns = [
        jax.jit(
            (lambda shape, dtype: (lambda: jnp.zeros(shape, dtype)))(
                (N_CORES * a.shape[0], *a.shape[1:]), a.dtype
            ),
            out_shardings=sharding,
        )
        for a in out_avals
    ]

    def _dev_input(nm, in_maps):
        arrs = [np.asarray(m[nm]) for m in in_maps]
        h = hashlib.blake2b(digest_size=16)
        for a in arrs:
            h.update(a.tobytes())
        key = (nm, h.hexdigest())
        if key not in dev_cache:
            if len(dev_cache) > 64:
                dev_cache.clear()
            dev_cache[key] = jax.device_put(
                np.concatenate(arrs, axis=0), sharding
            )
        return dev_cache[key]

    def run(in_maps):
        concat_in = [_dev_input(nm, in_maps) for nm in in_names]
        concat_zeros = [zf() for zf in zeros_fns]
        out_arrs = sharded(*concat_in, *concat_zeros)
        return [
            {
                nm: np.asarray(out_arrs[i]).reshape(N_CORES, *out_avals[i].shape)[c]
                for i, nm in enumerate(out_names)
            }
            for c in range(N_CORES)
        ]

    _CACHE["runner"] = run
    return run


def kernel(**inputs):
    run = _get_runner()
    in_maps = prep_inputs(**inputs)
    return assemble_output(run(in_maps))
